# revision 7
# baseline (speedup 1.0000x reference)
"""Sliding-window causal self-attention for Trainium2, 8 NeuronCores. v2.

Problem: B=4, T=2048, C=1024, 16 heads x 64 dim, window=256 causal band.
  qkv = x @ W_qkv.T ; windowed-causal attention ; out = y @ W_proj.T

Sharding v2: 8 cores = 4 batches x 2 head-halves (8 heads each).
Each core computes q/k/v for its 8 heads over the FULL sequence (no
halo recompute), runs attention for all 2048 queries, and produces a
PARTIAL projection output (its heads' contribution, [1024, 2048] f32).
The host sums the two partials per batch and transposes.

All matmuls in bf16 (1 cycle/row at any moving width on TRN2's PE):
  - half-width (128-col) triangle matmuls: the s0 (upper) and s3
    (lower) key-subtiles of each 256-query chunk are valid for
    complementary query halves, so they share one PSUM tile at half
    width -> 25% fewer score/AV cycles.
  - band masks are two 128x128 triangles (mU/mL) applied
    multiplicatively post-exp on DVE in bf16 (2x DVE rate).
  - exp on ScalarE reads each score subtile straight from PSUM
    ([128,512] both heads at once), writes PM bf16.
  - AV uses V augmented with a ones column per head (65 cols) so row 64
    of the PSUM y tile is the softmax denominator; normalization is a
    DVE tensor_tensor reading y from PSUM, multiplying by the broadcast
    reciprocal, writing bf16 y_sb in one pass.

Scheduling: a flat work-item list interleaves phase-A GEMM groups of
column-period t with the attention chunks of period t-1 (full lag), so
phase B's scalar/vector chains hide under phase A's PE-heavy stretches;
within a chunk, AV(hp) trails scores by two slots (lag-2) to cover the
exp->mask latency. Score subtiles flow sg2 -> sg1 -> sg03 and AV
consumes them in that order so the first AV matmul only waits on the
earliest exp. Phase-A accumulation groups, score tiles, AV/proj
accumulators all draw from one shared 8-bank PSUM ring.
"""

import numpy as np
import ml_dtypes
from contextlib import ExitStack

import concourse.bass as bass
import concourse.tile as tile
import concourse.mybir as mybir
from concourse import bacc
from concourse.tile import add_dep_helper
from concourse import bass_utils

F32 = mybir.dt.float32
BF16 = mybir.dt.bfloat16
AF = mybir.ActivationFunctionType

B = 4
T = 2048
C = 1024
HL = 8              # heads per core
D = 64
QC = 256            # queries per attention chunk
KI = C // 128       # 8 contraction blocks for qkv
NCH = T // QC       # 8 attention chunks
TCN = 4             # phase-A column chunks of 512


def _build_body(tc, xT, wqkvT, wprojT, masks, outT):
    nc = tc.nc
    with ExitStack() as ctx:
        kq_pool = ctx.enter_context(tc.tile_pool(name="kq", bufs=1))
        w_pool = ctx.enter_context(tc.tile_pool(name="ww", bufs=1))
        x_pool = ctx.enter_context(tc.tile_pool(name="xx", bufs=1))
        const_pool = ctx.enter_context(tc.tile_pool(name="const", bufs=1))
        pm_pool = ctx.enter_context(tc.tile_pool(name="pm", bufs=5))
        r_pool = ctx.enter_context(tc.tile_pool(name="rr", bufs=3))
        ysb_pool = ctx.enter_context(tc.tile_pool(name="ysb", bufs=2))
        o_pool = ctx.enter_context(tc.tile_pool(name="ost", bufs=3))
        psB = ctx.enter_context(tc.tile_pool(name="psB", bufs=8, space="PSUM"))

        # kZ: per-head zero-padded K. Head h occupies partition rows
        # [64*(h%2), 64*(h%2)+64); the sibling half stays zero so a score
        # matmul can contract over all 128 partitions at tile position
        # (0,0) -- mixing PE tile positions within one PSUM bank faults
        # the hardware. The paired qT moving operand is used unmodified:
        # the zero stationary half annihilates the other head's term.
        kZ = kq_pool.tile([128, HL, T], BF16)
        qT = kq_pool.tile([128, 4, T], BF16)
        V = kq_pool.tile([128, 16, HL * (D + 1)], BF16)
        xt = x_pool.tile([128, KI, T], BF16)
        wq_sb = w_pool.tile([128, KI, 1536], BF16)
        wp_sb = w_pool.tile([128, 4, 1024], BF16)
        masks_sb = const_pool.tile([128, 2, 128], BF16)

        nc.gpsimd.memset(kZ[:], 0.0)
        ones_col = const_pool.tile([128, 1], F32)
        nc.vector.memset(ones_col[:], 1.0)
        v_ones_view = V[:].rearrange("p e (h x) -> p e h x", x=D + 1)[:, :, :, D]
        nc.vector.tensor_copy(
            v_ones_view, ones_col[:, 0:1].broadcast_to([128, 16, HL])
        )

        # ---- input DMAs: q/k weight cols + x chunk 0 first (finest useful
        # grain so the first GEMM group starts ASAP), v cols next ----
        xTr = xT.rearrange("(o p) t -> p o t", p=128)
        wqr = wqkvT.rearrange("(o p) c -> p o c", p=128)
        nc.sync.dma_start(wq_sb[:, 0, 0:128], wqr[:, 0, 0:128])
        nc.sync.dma_start(xt[:, 0, 0:512], xTr[:, 0, 0:512])
        nc.sync.dma_start(wq_sb[:, 0, 128:1024], wqr[:, 0, 128:1024])
        for ki in range(1, KI):
            nc.sync.dma_start(wq_sb[:, ki, 0:1024], wqr[:, ki, 0:1024])
            nc.sync.dma_start(xt[:, ki, 0:512], xTr[:, ki, 0:512])
        nc.sync.dma_start(masks_sb[:], masks.rearrange("m p q -> p m q"))
        for ki in range(KI):
            nc.sync.dma_start(wq_sb[:, ki, 1024:1536], wqr[:, ki, 1024:1536])
        for t in range(1, TCN):
            nc.sync.dma_start(xt[:, :, 512 * t : 512 * t + 512],
                              xTr[:, :, 512 * t : 512 * t + 512])
        nc.sync.dma_start(wp_sb[:], wprojT.rearrange("(o p) c -> p o c", p=128))

        # ================= work items =================
        def a_group(t, kind, i):
            # one qkv accumulation group: q/k block co=i, or v rows eo
            def run():
                c0 = 512 * t
                ps = psB.tile([128, 512], F32, tag="bank", name=f"A{kind}{t}_{i}")
                if kind in ("q", "k"):
                    wof = 128 * i if kind == "q" else 512 + 128 * i
                    for ki in range(KI):
                        nc.tensor.matmul(
                            ps[:],
                            wq_sb[:, ki, wof : wof + 128],
                            xt[:, ki, c0 : c0 + 512],
                            start=(ki == 0),
                            stop=(ki == KI - 1),
                        )
                    if kind == "q":
                        nc.vector.tensor_copy(qT[:, i, c0 : c0 + 512], ps[:])
                    else:
                        nc.scalar.activation(
                            kZ[0:64, 2 * i, c0 : c0 + 512], ps[0:64, :], AF.Copy)
                        nc.scalar.activation(
                            kZ[64:128, 2 * i + 1, c0 : c0 + 512], ps[64:128, :],
                            AF.Copy)
                else:  # v, eo = i
                    eo = i
                    for ki in range(KI):
                        nc.tensor.matmul(
                            ps[:],
                            xt[:, ki, 128 * eo : 128 * eo + 128],
                            wq_sb[:, ki, 1024:1536],
                            start=(ki == 0),
                            stop=(ki == KI - 1),
                        )
                    v_dst = V[:].rearrange("p e (h x) -> p e h x", x=D + 1)[
                        :, eo, :, 0:D
                    ]
                    nc.scalar.activation(
                        v_dst, ps[:].rearrange("p (h d) -> p h d", d=D), AF.Copy
                    )
            return run

        state = {}  # (ch, hp) -> PM tile; pch -> y_sb tile

        def sc_item(ch, hp):
            # scores + exp + masks for head pair hp of chunk ch
            def run():
                q0 = QC * ch
                PM = pm_pool.tile([128, 3, 2, QC], BF16, tag="PM",
                                  name=f"PM{ch}_{hp}")
                state[(ch, hp)] = PM
                PMf = PM[:].rearrange("p s j q -> p s (j q)")
                masks_flat = masks_sb[:].rearrange("p m q -> p (m q)")
                # sg order: 2 (oldest-ready), 1, 0(=s03 triangles)
                sgl = [2, 1, 0] if ch > 0 else [2, 0]
                for sg in sgl:
                    s = psB.tile([128, 2, QC], F32, tag="bank",
                                 name=f"S{sg}_{ch}_{hp}")
                    first = None
                    for j in (0, 1):
                        if sg == 0:
                            if True:
                                # ch==0: dummy scores vs keys block 0; they
                                # are never consumed (ch0's AV skips s03a and
                                # cols 0:128 of PM sg0 are unread) but they
                                # keep the PSUM bank fully written so the exp
                                # read never touches another tile's region.
                                klo = 256 * ch - 256 if ch > 0 else 0
                                mm = nc.tensor.matmul(
                                    s[:, j, 0:128],
                                    kZ[:, 2 * hp + j, klo : klo + 128],
                                    qT[:, hp, q0 : q0 + 128],
                                    start=first is None, stop=True,
                                    skip_group_check=first is not None,
                                )
                                if first is not None:
                                    add_dep_helper(mm.ins, first.ins, sync=True,
                                                   reason="bank clear order")
                                first = first or mm
                            mm = nc.tensor.matmul(
                                s[:, j, 128:256],
                                kZ[:, 2 * hp + j,
                                   256 * ch + 128 : 256 * ch + 256],
                                qT[:, hp, q0 + 128 : q0 + 256],
                                start=first is None, stop=True,
                                skip_group_check=first is not None,
                            )
                            if first is not None:
                                add_dep_helper(mm.ins, first.ins, sync=True,
                                               reason="bank clear order")
                            first = first or mm
                        else:
                            kb = 2 * ch - 2 + sg
                            mm = nc.tensor.matmul(
                                s[:, j, :],
                                kZ[:, 2 * hp + j, 128 * kb : 128 * kb + 128],
                                qT[:, hp, q0 : q0 + QC],
                                start=first is None, stop=True,
                                skip_group_check=first is not None,
                            )
                            if first is not None:
                                add_dep_helper(mm.ins, first.ins, sync=True,
                                               reason="bank clear order")
                            first = first or mm
                    nc.scalar.activation(
                        PMf[:, sg, :],
                        s[:].rearrange("p j q -> p (j q)"),
                        AF.Exp, scale=0.125,
                    )
                    # masks for this subtile group
                    if sg == 0:
                        nc.vector.tensor_tensor(
                            out=PM[:, 0, :, :],
                            in0=PM[:, 0, :, :],
                            in1=masks_flat.unsqueeze(1).broadcast_to(
                                [128, 2, QC]),
                            op=mybir.AluOpType.mult,
                        )
                    elif sg == 1:
                        nc.vector.tensor_tensor(
                            out=PM[:, 1, :, 128:256],
                            in0=PM[:, 1, :, 128:256],
                            in1=masks_sb[:, 0, :].unsqueeze(1).broadcast_to(
                                [128, 2, 128]),
                            op=mybir.AluOpType.mult,
                        )
                    else:
                        nc.vector.tensor_tensor(
                            out=PM[:, 2, :, 0:128],
                            in0=PM[:, 2, :, 0:128],
                            in1=masks_sb[:, 1, :].unsqueeze(1).broadcast_to(
                                [128, 2, 128]),
                            op=mybir.AluOpType.mult,
                        )
            return run

        def avn_item(ch, hp):
            # AV + rowsum/recip/broadcast/normalize for (ch, hp)
            def run():
                PM = state.pop((ch, hp))
                y_sb = state[("y", ch // 2)]
                yb = psB.tile([128, 512], F32, tag="bank", name=f"yb{ch}_{hp}")
                first = None
                for j in (0, 1):
                    hl = 2 * hp + j
                    qof = 256 * j

                    # the first matmul (start=True, unskipped) marks the bank's
                    # group open; the last (stop=True, unskipped) closes it so
                    # the rowsum/normalize reads pass the sim's group check.
                    # Everything in between uses skip_group_check.
                    def vmm(kb, pm_ap, qlo, stop, last=False):
                        nonlocal first
                        mm = nc.tensor.matmul(
                            yb[0:65, qof + qlo : qof + qlo + 128],
                            V[:, kb, 65 * hl : 65 * hl + 65],
                            pm_ap,
                            start=first is None,
                            # group bookkeeping: opener keeps the group open,
                            # only the unskipped `last` matmul closes it
                            stop=stop and first is not None,
                            skip_group_check=(first is not None) and not last,
                        )
                        if first is not None:
                            add_dep_helper(mm.ins, first.ins, sync=True,
                                           reason="bank clear order")
                        first = first or mm

                    # consume in exp-completion order: s2, s1, s03
                    if ch > 0:
                        vmm(2 * ch, PM[:, 2, j, 0:128], 0, False)
                        vmm(2 * ch, PM[:, 2, j, 128:256], 128, False)
                        vmm(2 * ch - 1, PM[:, 1, j, 0:128], 0, False)
                        vmm(2 * ch - 1, PM[:, 1, j, 128:256], 128, False)
                        vmm(2 * ch - 2, PM[:, 0, j, 0:128], 0, True)
                        vmm(2 * ch + 1, PM[:, 0, j, 128:256], 128, True,
                            last=(j == 1))
                    else:
                        vmm(2 * ch, PM[:, 2, j, 0:128], 0, True)
                        vmm(2 * ch, PM[:, 2, j, 128:256], 128, False)
                        vmm(2 * ch + 1, PM[:, 0, j, 128:256], 128, True,
                            last=(j == 1))

                rt = r_pool.tile([1, 512], F32, tag="rt", name=f"rt{ch}{hp}")
                nc.scalar.activation(rt[:], yb[64:65, :], AF.Copy)
                Rr = r_pool.tile([1, 512], F32, tag="Rr", name=f"Rr{ch}{hp}")
                nc.vector.reciprocal_approx_fast(Rr[:], rt[0:1, :])
                rb = r_pool.tile([128, 512], F32, tag="rb", name=f"rb{ch}{hp}")
                nc.gpsimd.partition_broadcast(rb[:], Rr[0:1, :], channels=128)
                half = 256 * (ch % 2)
                for j in (0, 1):
                    nc.vector.tensor_tensor(
                        out=y_sb[64 * j : 64 * j + 64, hp, half : half + 256],
                        in0=yb[0:64, 256 * j : 256 * j + 256],
                        in1=rb[64 * j : 64 * j + 64, 256 * j : 256 * j + 256],
                        op=mybir.AluOpType.mult,
                    )
            return run

        def ysb_item(pch):
            def run():
                state[("y", pch)] = ysb_pool.tile(
                    [128, 4, 512], BF16, tag="ysb", name=f"ysb{pch}")
            return run

        outTr = outT.rearrange("(o p) t -> p o t", p=128)

        def pj_parts(pch):
            # projection for query cols [512*pch, 512*pch+512) as four
            # 2-co-block sub-items (usable as PE gap fillers); all 8 blocks
            # stage into one tile and ship as two batched DMAs so the
            # out-DMA path pays 2x instead of 8x the per-DMA DGE overhead
            ob = [None]

            def part(h):
                def run():
                    if h == 0:
                        ob[0] = o_pool.tile([128, 8, 512], BF16, tag="o",
                                            name=f"ob{pch}")
                    y_sb = state[("y", pch)]
                    for co in (2 * h, 2 * h + 1):
                        pp = psB.tile([128, 512], F32, tag="bank",
                                      name=f"pp{pch}{co}")
                        for ci in range(4):
                            nc.tensor.matmul(
                                pp[:],
                                wp_sb[:, ci, 128 * co : 128 * co + 128],
                                y_sb[:, ci, :],
                                start=(ci == 0),
                                stop=(ci == 3),
                            )
                        if co % 2 == 0:
                            nc.scalar.activation(ob[0][:, co, :], pp[:], AF.Copy)
                        else:
                            nc.vector.tensor_copy(ob[0][:, co, :], pp[:])
                    if h == 1 or h == 3:
                        nc.sync.dma_start(
                            outTr[:, 4 * (h // 2) : 4 * (h // 2) + 4,
                                  512 * pch : 512 * pch + 512],
                            ob[0][:, 4 * (h // 2) : 4 * (h // 2) + 4, :],
                        )
                return run

            return [part(h) for h in range(4)]

        def pj_item(pch):
            parts = pj_parts(pch)

            def run():
                for p in parts:
                    p()
            return run

        def pj_half(pch, ob, qlo):
            # 256-col half of a projection pass (gated by one chunk's norms
            # only); all 8 co blocks, one batched out-DMA
            def run():
                if ob[0] is None:
                    ob[0] = o_pool.tile([128, 8, 512], BF16, tag="o",
                                        name=f"obh{pch}")
                y_sb = state[("y", pch)]
                for co in range(8):
                    pp = psB.tile([128, 512], F32, tag="bank",
                                  name=f"ph{pch}{co}{qlo}")
                    for ci in range(4):
                        nc.tensor.matmul(
                            pp[:, 0:256],
                            wp_sb[:, ci, 128 * co : 128 * co + 128],
                            y_sb[:, ci, qlo : qlo + 256],
                            start=(ci == 0),
                            stop=(ci == 3),
                        )
                    if co == 7:
                        # last block: halves on both engines so the terminal
                        # copy is half as long
                        nc.scalar.activation(ob[0][:, co, qlo : qlo + 128],
                                             pp[:, 0:128], AF.Copy)
                        nc.vector.tensor_copy(ob[0][:, co, qlo + 128 : qlo + 256],
                                              pp[:, 128:256])
                    elif co % 2 == 0:
                        nc.scalar.activation(ob[0][:, co, qlo : qlo + 256],
                                             pp[:, 0:256], AF.Copy)
                    else:
                        nc.vector.tensor_copy(ob[0][:, co, qlo : qlo + 256],
                                              pp[:, 0:256])
                    if co == 3:
                        nc.sync.dma_start(
                            outTr[:, 0:4, 512 * pch + qlo : 512 * pch + qlo + 256],
                            ob[0][:, 0:4, qlo : qlo + 256],
                        )
                nc.sync.dma_start(
                    outTr[:, 4:8, 512 * pch + qlo : 512 * pch + qlo + 256],
                    ob[0][:, 4:8, qlo : qlo + 256],
                )
            return run

        # ---- full-lag schedule: period t emits phase-A columns t
        # interleaved with the attention chunks of period t-1 (whose
        # dependencies are long satisfied), so B's scalar/vector chains
        # hide under A's PE-heavy stretch. Within the B stream, AV(hp)
        # trails scores(hp+1) by one slot (lag-1). ----
        def a_items_for(t):
            items = []
            for co in range(4):
                items.append(a_group(t, "q", co))
            for co in range(4):
                items.append(a_group(t, "k", co))
            for eo in range(4 * t, 4 * t + 4):
                items.append(a_group(t, "v", eo))
            return items

        def b_items_for(c0, c1, lag=2):
            # lag-N pipeline over the 8 (ch, hp) slots of chunks c0, c1
            slots = [(c0, hp) for hp in range(4)] + [(c1, hp) for hp in range(4)]
            items = [ysb_item(c0 // 2)]
            pend = []
            for s in slots:
                items.append(sc_item(*s))
                pend.append(s)
                if len(pend) > lag:
                    items.append(avn_item(*pend.pop(0)))
            for s in pend:
                items.append(avn_item(*s))
            return items

        def interleave(a_items, b_items):
            na, nb = len(a_items), len(b_items)
            ia = ib = 0
            while ia < na or ib < nb:
                if ia < na and (ib >= nb or ia * nb <= ib * na):
                    a_items[ia]()
                    ia += 1
                else:
                    b_items[ib]()
                    ib += 1

        # period 0: phase A t=0 alone
        interleave(a_items_for(0), [])
        # periods 1..3: A(t) x B(2t-2), B(2t-1) [+ proj of the pair before]
        for t in range(1, 4):
            b_items = b_items_for(2 * t - 2, 2 * t - 1)
            if t >= 2:
                # a few slots in so the freshest norm (its last-ci input)
                # clears before the proj matmuls reach PE
                b_items.insert(2, pj_item(t - 2))
            if t == 3:
                # preheat the tail: chunk 6's first two score items run
                # under A3's GEMM cover (their kT/qT columns come from
                # A3's own q/k groups, emitted earlier in this period)
                b_items += [sc_item(6, 0), sc_item(6, 1)]
            interleave(a_items_for(t), b_items)
        # tail: chunks 6,7; pj2's four sub-items are sprinkled between the
        # attention slots as PE fillers for the exp->mask latency that no
        # phase-A work is left to hide. pj3 runs in half-chunk passes: the
        # chunk-6 half overlaps chunk 7's attention, so only the 256-col
        # chunk-7 half (and one 0.5MB DMA) trails the last norm.
        p2 = pj_parts(2)
        ob3 = [None]
        sc, avn = sc_item, avn_item
        tail = [
            ysb_item(3), sc(6, 2), avn(6, 0),
            sc(6, 3), avn(6, 1), p2[0], sc(7, 0), avn(6, 2), p2[1],
            sc(7, 1), avn(6, 3), p2[2], sc(7, 2), avn(7, 0), p2[3],
            sc(7, 3), avn(7, 1), pj_half(3, ob3, 0),
            avn(7, 2), avn(7, 3), pj_half(3, ob3, 256),
        ]
        for it in tail:
            it()


_BUILD_CACHE = {}


def build_bass(enable_asserts=False):
    key = enable_asserts
    if key in _BUILD_CACHE:
        return _BUILD_CACHE[key]
    nc = bacc.Bacc(
        "TRN2",
        target_bir_lowering=False,
        debug=False,
        enable_asserts=enable_asserts,
    )
    xT = nc.dram_tensor("xT", [C, T], BF16, kind="ExternalInput").ap()
    wqkvT = nc.dram_tensor("wqkvT", [C, 1536], BF16, kind="ExternalInput").ap()
    wprojT = nc.dram_tensor("wprojT", [512, C], BF16, kind="ExternalInput").ap()
    masks = nc.dram_tensor("masks", [2, 128, 128], BF16, kind="ExternalInput").ap()
    outT = nc.dram_tensor("outT", [C, T], BF16, kind="ExternalOutput").ap()

    with tile.TileContext(nc) as tc:
        _build_body(tc, xT, wqkvT, wprojT, masks, outT)
    nc.compile()
    _BUILD_CACHE[key] = nc
    return nc


def make_masks() -> np.ndarray:
    """[2, 128, 128]: mU[kr,qq] = kr >= qq+1 (s0/s1-edge), mL = kr <= qq
    (s3/s2-edge)."""
    kr = np.arange(128)[:, None]
    qq = np.arange(128)[None, :]
    mU = (kr >= qq + 1).astype(np.float32)
    mL = (kr <= qq).astype(np.float32)
    return np.stack([mU, mL]).astype(ml_dtypes.bfloat16)


def kernel(x, W_qkv, W_proj):
    bf16 = ml_dtypes.bfloat16
    x = np.asarray(x, dtype=np.float32)
    W_qkv = np.asarray(W_qkv, dtype=np.float32)
    W_proj = np.asarray(W_proj, dtype=np.float32)

    nc = build_bass()
    wqkvT = W_qkv.T  # [C, 3C]
    wprojT = W_proj.T  # [C, C] (in, out)
    masks_np = make_masks()

    xTb = [np.ascontiguousarray(x[b].T).astype(bf16) for b in range(B)]
    in_maps = []
    for core in range(8):
        b, hh = core // 2, core % 2
        sl = slice(512 * hh, 512 * hh + 512)
        wq = np.concatenate(
            [wqkvT[:, sl], wqkvT[:, 1024 + 512 * hh : 1536 + 512 * hh],
             wqkvT[:, 2048 + 512 * hh : 2560 + 512 * hh]], axis=1
        ).astype(bf16)
        in_maps.append({
            "xT": xTb[b],
            "wqkvT": np.ascontiguousarray(wq),
            "wprojT": np.ascontiguousarray(wprojT[sl, :]).astype(bf16),
            "masks": masks_np,
        })
    res = bass_utils.run_bass_kernel_spmd(nc, in_maps, core_ids=list(range(8)))
    kernel.last_run_results = res

    out = np.empty((B, T, C), dtype=np.float32)
    for b in range(B):
        out[b] = (res.results[2 * b]["outT"].astype(np.float32)
                  + res.results[2 * b + 1]["outT"].astype(np.float32)).T
    return out


kernel.last_run_results = None
